# revision 8
# baseline (speedup 1.0000x reference)
"""Bass/Tile Trainium2 kernel for the additive-attention module.

reference (per batch row b):
    q = hidden_state @ Wa.T + ba                 # [A]
    k = feature_vectors[b] @ Ua.T                # [L, A]
    e = tanh(q + k) @ w                          # [L]
    attn = softmax(e)                            # [L]
    context[b] = attn @ feature_vectors[b]       # [M]

Sharding: data-parallel over batch B=64 -> 8 cores x 8 rows, params
replicated, no collectives. Each core streams its 32 MB feature_vector
shard from HBM exactly once.

Precision: fp32 matmuls cost 4 cycles/column on TRN2 PE (two half-rate
passes); fp16 costs 1. The fv pipeline (fv, Ua, tanh output, attn
weights) runs in fp16 (10 mantissa bits; measured end-to-end rel err
2.7e-3 vs 1.8e-2 for bf16); softmax statistics and all accumulations
(PSUM) stay fp32.

Per-core dataflow (per batch row):
  - fv cast fp32->fp16 during the HBM DMA (SWDGE), natural [l, m] layout
  - PE transposes 128x128 fp16 tiles of fv into [m, l] layout (PSUM),
    DVE evacuates at 16-bit 2x mode; k-matmul streams fv.T columns with
    Ua.T stationary
  - ScalarE evacuates the k PSUM with fused per-partition bias q[a] and
    tanh in one ACTIVATE (fp16 out)
  - e = w.T @ tanh(...) on PE; softmax: DRAM-bounce reshape of e
    [1,4096] -> [128,32], DVE row max, GPSIMD cross-partition max, ACT
    exp with accum_out row sums, GPSIMD cross-partition sum
  - weighted sum on PE: attn column [128,1] fp16 stationary, natural
    fv tiles streaming; denominator applied at the end in fp32
  - the weighted-sum stage of row b is emitted after row b+1's main
    stage so PE never stalls on the softmax tail
"""

import numpy as np

B, R, M, A, L = 64, 512, 256, 256, 4096
NCORES = 8
BLOC = B // NCORES  # 8 batch rows per core
NL = L // 128  # 32 l-chunks of 128
NJG = 8  # j-groups of 512 l-columns
JW = L // NJG  # 512

_CACHE = {}


def _build():
    from contextlib import ExitStack

    import concourse.bacc as bacc
    import concourse.bass as bass
    import concourse.bass_isa as bass_isa
    import concourse.mybir as mybir
    import concourse.tile as tile
    from concourse.masks import make_identity

    f32 = mybir.dt.float32
    f16 = mybir.dt.float16
    AF = mybir.ActivationFunctionType

    nc = bacc.Bacc("TRN2", target_bir_lowering=False, debug=False,
                   num_devices=NCORES)

    hs = nc.dram_tensor("hidden_state", [BLOC, R], f32, kind="ExternalInput").ap()
    fv = nc.dram_tensor("feature_vectors", [BLOC, L, M], f32,
                        kind="ExternalInput").ap()
    Wa = nc.dram_tensor("Wa", [A, R], f32, kind="ExternalInput").ap()
    Ua = nc.dram_tensor("Ua", [A, M], f32, kind="ExternalInput").ap()
    w = nc.dram_tensor("w", [A, 1], f32, kind="ExternalInput").ap()
    ba = nc.dram_tensor("ba", [1, A], f32, kind="ExternalInput").ap()
    ctx_out = nc.dram_tensor("context", [BLOC, M], f32, kind="ExternalOutput").ap()

    with tile.TileContext(nc) as tc, ExitStack() as ctx:
        singles = ctx.enter_context(tc.tile_pool(name="singles", bufs=1))
        ldpool = ctx.enter_context(tc.tile_pool(name="ldpool", bufs=2))
        fvpool = ctx.enter_context(tc.tile_pool(name="fvpool", bufs=3))
        work = ctx.enter_context(tc.tile_pool(name="work", bufs=3))
        small = ctx.enter_context(tc.tile_pool(name="small", bufs=2))
        ps_tp = ctx.enter_context(tc.tile_pool(name="ps_tp", bufs=2, space="PSUM"))
        ps_k = ctx.enter_context(tc.tile_pool(name="ps_k", bufs=3, space="PSUM"))
        ps_e = ctx.enter_context(tc.tile_pool(name="ps_e", bufs=2, space="PSUM"))
        ps_mm = ctx.enter_context(tc.tile_pool(name="ps_mm", bufs=1, space="PSUM"))
        dram = ctx.enter_context(tc.tile_pool(name="dram", bufs=2, space="DRAM"))

        # batch 0's fv load goes first: it shares the GPSIMD SWDGE queue
        # with make_identity and must start as early as possible
        fv_loaded = {}

        def load_fv(b):
            # 4 cast-DMAs (SWDGE f32->f16) of 8 l-chunks each so the first
            # transposes start ~3 us after the load begins, not ~12 us
            fv_nat = fvpool.tile([128, NL, M], f16, tag="fv", name="fv")
            for part in range(4):
                src = bass.AP(tensor=fv.tensor,
                              offset=b * L * M + part * (NL // 4) * 128 * M,
                              ap=[[M, 128], [128 * M, NL // 4], [1, M]])
                nc.gpsimd.dma_start(out=fv_nat[:, part * (NL // 4):(part + 1) * (NL // 4), :],
                                    in_=src)
            fv_loaded[b] = fv_nat
            return fv_nat

        load_fv(0)

        ident = singles.tile([128, 128], f32, tag="ident", name="ident")
        make_identity(nc, ident)
        ident16 = singles.tile([128, 128], f16, tag="ident16", name="ident16")
        make_identity(nc, ident16)

        # ---- parameters into contraction-major layouts ----
        # WaT[rt] [128(r), 256(a)] fp32: WaT[rt][k, a] = Wa[a, 128*rt + k]
        WaT = [singles.tile([128, A], f32, tag=f"WaT{rt}", name=f"WaT{rt}")
               for rt in range(4)]
        for at in range(2):
            wa_nat = ldpool.tile([128, R], f32, tag="ld", name="ld")
            nc.sync.dma_start(out=wa_nat, in_=Wa[at * 128:(at + 1) * 128, :])
            for rt in range(4):
                ps = ps_mm.tile([128, 128], f32, tag="mm", name="mm")
                nc.tensor.transpose(ps, wa_nat[:, rt * 128:(rt + 1) * 128], ident)
                nc.vector.tensor_copy(out=WaT[rt][:, at * 128:(at + 1) * 128],
                                      in_=ps)
        # UaT[mh] [128(m), 256(a)] fp16: UaT[mh][k, a] = Ua[a, 128*mh + k]
        UaT = [singles.tile([128, A], f16, tag=f"UaT{mh}", name=f"UaT{mh}")
               for mh in range(2)]
        for at in range(2):
            ua_nat = ldpool.tile([128, M], f32, tag="ld", name="ld")
            nc.sync.dma_start(out=ua_nat, in_=Ua[at * 128:(at + 1) * 128, :])
            for mh in range(2):
                ps = ps_mm.tile([128, 128], f32, tag="mm", name="mm")
                nc.tensor.transpose(ps, ua_nat[:, mh * 128:(mh + 1) * 128], ident)
                nc.vector.tensor_copy(out=UaT[mh][:, at * 128:(at + 1) * 128],
                                      in_=ps)
        # w as fp16 stationary columns [128, 1] per a-half (cast during DMA)
        w_sb = [singles.tile([128, 1], f16, tag=f"w{ah}", name=f"w{ah}")
                for ah in range(2)]
        for ah in range(2):
            nc.gpsimd.dma_start(out=w_sb[ah], in_=w[ah * 128:(ah + 1) * 128, :])

        # hsT[rt] [128(r), BLOC] fp32
        hsT = [singles.tile([128, BLOC], f32, tag=f"hsT{rt}", name=f"hsT{rt}")
               for rt in range(4)]
        for rt in range(4):
            src = bass.AP(tensor=hs.tensor, offset=rt * 128,
                          ap=[[1, 128], [R, BLOC]])
            nc.sync.dma_start(out=hsT[rt], in_=src)

        # q = hs @ Wa.T + ba   -> [BLOC, A] fp32
        q_ps = ps_mm.tile([BLOC, A], f32, tag="mm", name="mm")
        for rt in range(4):
            nc.tensor.matmul(q_ps, lhsT=hsT[rt], rhs=WaT[rt],
                             start=(rt == 0), stop=(rt == 3))
        ba_b = singles.tile([BLOC, A], f32, tag="ba", name="ba")
        nc.sync.dma_start(out=ba_b,
                          in_=bass.AP(tensor=ba.tensor, offset=0,
                                      ap=[[0, BLOC], [1, A]]))
        q_sb = singles.tile([BLOC, A], f32, tag="q", name="q")
        nc.vector.tensor_add(q_sb, q_ps, ba_b)
        # qT[ah] [128(a), BLOC] fp32
        qT = [singles.tile([128, BLOC], f32, tag=f"qT{ah}", name=f"qT{ah}")
              for ah in range(2)]
        for ah in range(2):
            ps = ps_mm.tile([128, BLOC], f32, tag="mm", name="mm")
            nc.tensor.transpose(ps, q_sb[:, ah * 128:(ah + 1) * 128],
                                ident[:BLOC, :BLOC])
            nc.vector.tensor_copy(out=qT[ah], in_=ps)

        # ---- main per-batch-row pipeline ----
        def main_stage(b):
            fv_nat = fv_loaded.pop(b) if b in fv_loaded else load_fv(b)

            e_sb = small.tile([1, L], f32, tag="e_sb", name="e_sb")

            # software-pipelined j-groups: PE emits transposes(i),
            # k-matmuls(i-1), e-matmul(i-2) per step so it never stalls on
            # the DVE fvT-copy or ACT tanh of the current group.
            fvT_q = {}
            t_q = {}

            def emit_T(jg):
                # both m-halves' transposes share one fp16 PSUM bank
                pst = ps_tp.tile([128, 2, JW], f16, tag="tp", name="tp")
                for mh in range(2):
                    for c in range(4):
                        t = jg * 4 + c
                        nc.tensor.transpose(
                            pst[:, mh, c * 128:(c + 1) * 128],
                            fv_nat[:, t, mh * 128:(mh + 1) * 128], ident16)
                fvT = [work.tile([128, JW], f16, tag=f"fvT{mh}", name=f"fvT{mh}")
                       for mh in range(2)]
                for mh in range(2):
                    nc.vector.tensor_copy(out=fvT[mh], in_=pst[:, mh, :])
                fvT_q[jg] = fvT

            def emit_K(jg):
                fvT = fvT_q.pop(jg)
                t_sb = [work.tile([128, JW], f16, tag=f"t{ah}", name=f"t{ah}")
                        for ah in range(2)]
                for ah in range(2):
                    psk = ps_k.tile([128, JW], f32, tag="kk", name="kk")
                    for mh in range(2):
                        nc.tensor.matmul(
                            psk, lhsT=UaT[mh][:, ah * 128:(ah + 1) * 128],
                            rhs=fvT[mh], start=(mh == 0), stop=(mh == 1))
                    nc.scalar.activation(out=t_sb[ah], in_=psk, func=AF.Tanh,
                                         bias=qT[ah][:, b:b + 1], scale=1.0)
                t_q[jg] = t_sb

            def emit_E(jg):
                t_sb = t_q.pop(jg)
                pse = ps_e.tile([1, JW], f32, tag="ee", name="ee")
                for ah in range(2):
                    nc.tensor.matmul(pse, lhsT=w_sb[ah], rhs=t_sb[ah],
                                     start=(ah == 0), stop=(ah == 1))
                nc.vector.tensor_copy(out=e_sb[:, jg * JW:(jg + 1) * JW],
                                      in_=pse)

            for i in range(NJG + 2):
                if i < NJG:
                    emit_T(i)
                if 1 <= i:
                    emit_K(i - 1) if (i - 1) < NJG else None
                if 2 <= i:
                    emit_E(i - 2)

            # softmax pieces: e [1, 4096] -> e_t [128, 32] with
            # e_t[p, t] = e[128*t + p] (partition scatter via DRAM bounce)
            e_t = small.tile([128, NL], f32, tag="e_t", name="e_t")
            e_d = dram.tile([L], f32, tag="e_d", name="e_d")
            nc.sync.dma_start(
                out=bass.AP(tensor=e_d.tensor, offset=e_d.offset,
                            ap=[[0, 1], [1, L]]),
                in_=e_sb)
            nc.sync.dma_start(
                out=e_t,
                in_=bass.AP(tensor=e_d.tensor, offset=e_d.offset,
                            ap=[[1, 128], [128, NL]]))
            mrow = small.tile([128, 1], f32, tag="mrow", name="mrow")
            nc.vector.reduce_max(out=mrow, in_=e_t, axis=mybir.AxisListType.X)
            mall = small.tile([128, 1], f32, tag="mall", name="mall")
            nc.gpsimd.partition_all_reduce(mall, mrow, channels=128,
                                           reduce_op=bass_isa.ReduceOp.max)
            negm = small.tile([128, 1], f32, tag="negm", name="negm")
            nc.vector.tensor_scalar_mul(negm, mall, -1.0)
            p_t = small.tile([128, NL], f16, tag="p_t", name="p_t")
            srow = small.tile([128, 1], f32, tag="srow", name="srow")
            nc.scalar.activation(out=p_t, in_=e_t, func=AF.Exp, bias=negm,
                                 scale=1.0, accum_out=srow)
            sall = small.tile([128, 1], f32, tag="sall", name="sall")
            nc.gpsimd.partition_all_reduce(sall, srow, channels=128,
                                           reduce_op=bass_isa.ReduceOp.add)
            rz = small.tile([1, 1], f32, tag="rz", name="rz")
            nc.vector.reciprocal(out=rz, in_=sall[0:1, :])
            return fv_nat, p_t, rz

        # weighted sum, one batch row behind
        def ws_stage(b, fv_nat, p_t, rz):
            psw = ps_mm.tile([1, M], f32, tag="mm", name="mm")
            for t in range(NL):
                nc.tensor.matmul(psw, lhsT=p_t[:, t:t + 1], rhs=fv_nat[:, t, :],
                                 start=(t == 0), stop=(t == NL - 1))
            ctxs = small.tile([1, M], f32, tag="ctx", name="ctx")
            nc.vector.tensor_scalar_mul(ctxs, psw, rz)
            nc.sync.dma_start(out=ctx_out[b:b + 1, :], in_=ctxs)

        pending = None
        for b in range(BLOC):
            saved = main_stage(b)
            if pending is not None:
                ws_stage(pending[0], *pending[1])
            pending = (b, saved)
        ws_stage(pending[0], *pending[1])

    nc.compile()
    return nc


def _get_nc():
    if "nc" not in _CACHE:
        _CACHE["nc"] = _build()
    return _CACHE["nc"]


def kernel(hidden_state, feature_vectors, Wa, Ua, w, ba):
    from concourse.bass_utils import run_bass_kernel_spmd

    nc = _get_nc()
    hidden_state = np.ascontiguousarray(hidden_state, dtype=np.float32)
    feature_vectors = np.ascontiguousarray(feature_vectors, dtype=np.float32)
    params = {
        "Wa": np.ascontiguousarray(Wa, dtype=np.float32),
        "Ua": np.ascontiguousarray(Ua, dtype=np.float32),
        "w": np.ascontiguousarray(w, dtype=np.float32),
        "ba": np.ascontiguousarray(ba, dtype=np.float32),
    }
    in_maps = [
        {
            "hidden_state": hidden_state[c * BLOC:(c + 1) * BLOC],
            "feature_vectors": feature_vectors[c * BLOC:(c + 1) * BLOC],
            **params,
        }
        for c in range(NCORES)
    ]
    res = run_bass_kernel_spmd(nc, in_maps, list(range(NCORES)))
    return np.concatenate([res.results[c]["context"] for c in range(NCORES)],
                          axis=0)


# revision 9
# speedup vs baseline: 1.1045x; 1.1045x over previous
"""Bass/Tile Trainium2 kernel for the additive-attention module.

reference (per batch row b):
    q = hidden_state @ Wa.T + ba                 # [A]
    k = feature_vectors[b] @ Ua.T                # [L, A]
    e = tanh(q + k) @ w                          # [L]
    attn = softmax(e)                            # [L]
    context[b] = attn @ feature_vectors[b]       # [M]

Sharding: data-parallel over batch B=64 -> 8 cores x 8 rows, params
replicated, no collectives. Each core streams its 32 MB feature_vector
shard from HBM exactly once.

Precision: fp32 matmuls cost 4 cycles/column on TRN2 PE (two half-rate
passes); fp16 costs 1. The fv pipeline (fv, Ua, tanh output, attn
weights) runs in fp16 (10 mantissa bits; measured end-to-end rel err
2.7e-3 vs 1.8e-2 for bf16); softmax statistics and all accumulations
(PSUM) stay fp32.

Per-core dataflow (per batch row):
  - fv cast fp32->fp16 during the HBM DMA (SWDGE), natural [l, m] layout
  - PE transposes 128x128 fp16 tiles of fv into [m, l] layout (PSUM),
    DVE evacuates at 16-bit 2x mode; k-matmul streams fv.T columns with
    Ua.T stationary
  - ScalarE evacuates the k PSUM with fused per-partition bias q[a] and
    tanh in one ACTIVATE (fp16 out)
  - e = w.T @ tanh(...) on PE; softmax: DRAM-bounce reshape of e
    [1,4096] -> [128,32], DVE row max, GPSIMD cross-partition max, ACT
    exp with accum_out row sums, GPSIMD cross-partition sum
  - weighted sum on PE: attn column [128,1] fp16 stationary, natural
    fv tiles streaming; denominator applied at the end in fp32
  - the weighted-sum stage of row b is emitted after row b+1's main
    stage so PE never stalls on the softmax tail
"""

import numpy as np

B, R, M, A, L = 64, 512, 256, 256, 4096
NCORES = 8
BLOC = B // NCORES  # 8 batch rows per core
NL = L // 128  # 32 l-chunks of 128
NJG = 8  # j-groups of 512 l-columns
JW = L // NJG  # 512

_CACHE = {}


def _build():
    from contextlib import ExitStack

    import concourse.bacc as bacc
    import concourse.bass as bass
    import concourse.bass_isa as bass_isa
    import concourse.mybir as mybir
    import concourse.tile as tile
    from concourse.masks import make_identity

    f32 = mybir.dt.float32
    f16 = mybir.dt.float16
    AF = mybir.ActivationFunctionType

    nc = bacc.Bacc("TRN2", target_bir_lowering=False, debug=False,
                   num_devices=NCORES)

    hs = nc.dram_tensor("hidden_state", [BLOC, R], f32, kind="ExternalInput").ap()
    fv = nc.dram_tensor("feature_vectors", [BLOC, L, M], f32,
                        kind="ExternalInput").ap()
    Wa = nc.dram_tensor("Wa", [A, R], f32, kind="ExternalInput").ap()
    Ua = nc.dram_tensor("Ua", [A, M], f32, kind="ExternalInput").ap()
    w = nc.dram_tensor("w", [A, 1], f32, kind="ExternalInput").ap()
    ba = nc.dram_tensor("ba", [1, A], f32, kind="ExternalInput").ap()
    ctx_out = nc.dram_tensor("context", [BLOC, M], f32, kind="ExternalOutput").ap()

    with tile.TileContext(nc) as tc, ExitStack() as ctx:
        singles = ctx.enter_context(tc.tile_pool(name="singles", bufs=1))
        ldpool = ctx.enter_context(tc.tile_pool(name="ldpool", bufs=2))
        fvpool = ctx.enter_context(tc.tile_pool(name="fvpool", bufs=3))
        work = ctx.enter_context(tc.tile_pool(name="work", bufs=3))
        small = ctx.enter_context(tc.tile_pool(name="small", bufs=2))
        ps_tp = ctx.enter_context(tc.tile_pool(name="ps_tp", bufs=2, space="PSUM"))
        ps_k = ctx.enter_context(tc.tile_pool(name="ps_k", bufs=3, space="PSUM"))
        ps_e = ctx.enter_context(tc.tile_pool(name="ps_e", bufs=2, space="PSUM"))
        ps_mm = ctx.enter_context(tc.tile_pool(name="ps_mm", bufs=1, space="PSUM"))
        dram = ctx.enter_context(tc.tile_pool(name="dram", bufs=2, space="DRAM"))

        ident = singles.tile([128, 128], f32, tag="ident", name="ident")
        make_identity(nc, ident)
        ident16 = singles.tile([128, 128], f16, tag="ident16", name="ident16")
        make_identity(nc, ident16)

        # ---- parameters into contraction-major layouts ----
        # WaT[rt] [128(r), 256(a)] fp32: WaT[rt][k, a] = Wa[a, 128*rt + k]
        WaT = [singles.tile([128, A], f32, tag=f"WaT{rt}", name=f"WaT{rt}")
               for rt in range(4)]
        for at in range(2):
            wa_nat = ldpool.tile([128, R], f32, tag="ld", name="ld")
            nc.sync.dma_start(out=wa_nat, in_=Wa[at * 128:(at + 1) * 128, :])
            for rt in range(4):
                ps = ps_mm.tile([128, 128], f32, tag="mm", name="mm")
                nc.tensor.transpose(ps, wa_nat[:, rt * 128:(rt + 1) * 128], ident)
                nc.vector.tensor_copy(out=WaT[rt][:, at * 128:(at + 1) * 128],
                                      in_=ps)
        # UaT[mh] [128(m), 256(a)] fp16: UaT[mh][k, a] = Ua[a, 128*mh + k]
        UaT = [singles.tile([128, A], f16, tag=f"UaT{mh}", name=f"UaT{mh}")
               for mh in range(2)]
        for at in range(2):
            ua_nat = ldpool.tile([128, M], f32, tag="ld", name="ld")
            nc.sync.dma_start(out=ua_nat, in_=Ua[at * 128:(at + 1) * 128, :])
            for mh in range(2):
                ps = ps_mm.tile([128, 128], f32, tag="mm", name="mm")
                nc.tensor.transpose(ps, ua_nat[:, mh * 128:(mh + 1) * 128], ident)
                nc.vector.tensor_copy(out=UaT[mh][:, at * 128:(at + 1) * 128],
                                      in_=ps)
        # w as fp16 stationary columns [128, 1] per a-half (cast during DMA)
        w_sb = [singles.tile([128, 1], f16, tag=f"w{ah}", name=f"w{ah}")
                for ah in range(2)]
        for ah in range(2):
            nc.gpsimd.dma_start(out=w_sb[ah], in_=w[ah * 128:(ah + 1) * 128, :])

        # hsT[rt] [128(r), BLOC] fp32
        hsT = [singles.tile([128, BLOC], f32, tag=f"hsT{rt}", name=f"hsT{rt}")
               for rt in range(4)]
        for rt in range(4):
            src = bass.AP(tensor=hs.tensor, offset=rt * 128,
                          ap=[[1, 128], [R, BLOC]])
            nc.sync.dma_start(out=hsT[rt], in_=src)

        # q = hs @ Wa.T + ba   -> [BLOC, A] fp32
        q_ps = ps_mm.tile([BLOC, A], f32, tag="mm", name="mm")
        for rt in range(4):
            nc.tensor.matmul(q_ps, lhsT=hsT[rt], rhs=WaT[rt],
                             start=(rt == 0), stop=(rt == 3))
        ba_b = singles.tile([BLOC, A], f32, tag="ba", name="ba")
        nc.sync.dma_start(out=ba_b,
                          in_=bass.AP(tensor=ba.tensor, offset=0,
                                      ap=[[0, BLOC], [1, A]]))
        q_sb = singles.tile([BLOC, A], f32, tag="q", name="q")
        nc.vector.tensor_add(q_sb, q_ps, ba_b)
        # qT[ah] [128(a), BLOC] fp32
        qT = [singles.tile([128, BLOC], f32, tag=f"qT{ah}", name=f"qT{ah}")
              for ah in range(2)]
        for ah in range(2):
            ps = ps_mm.tile([128, BLOC], f32, tag="mm", name="mm")
            nc.tensor.transpose(ps, q_sb[:, ah * 128:(ah + 1) * 128],
                                ident[:BLOC, :BLOC])
            nc.vector.tensor_copy(out=qT[ah], in_=ps)

        # ---- main per-batch-row pipeline ----
        def main_stage(b):
            fv_nat = fvpool.tile([128, NL, M], f16, tag="fv", name="fv")
            src = bass.AP(tensor=fv.tensor, offset=b * L * M,
                          ap=[[M, 128], [128 * M, NL], [1, M]])
            nc.gpsimd.dma_start(out=fv_nat, in_=src)  # SWDGE cast f32->f16

            e_sb = small.tile([1, L], f32, tag="e_sb", name="e_sb")

            # software-pipelined j-groups: PE emits transposes(i),
            # k-matmuls(i-1), e-matmul(i-2) per step so it never stalls on
            # the DVE fvT-copy or ACT tanh of the current group.
            fvT_q = {}
            t_q = {}

            def emit_T(jg):
                # both m-halves' transposes share one fp16 PSUM bank
                pst = ps_tp.tile([128, 2, JW], f16, tag="tp", name="tp")
                for mh in range(2):
                    for c in range(4):
                        t = jg * 4 + c
                        nc.tensor.transpose(
                            pst[:, mh, c * 128:(c + 1) * 128],
                            fv_nat[:, t, mh * 128:(mh + 1) * 128], ident16)
                fvT = [work.tile([128, JW], f16, tag=f"fvT{mh}", name=f"fvT{mh}")
                       for mh in range(2)]
                for mh in range(2):
                    nc.vector.tensor_copy(out=fvT[mh], in_=pst[:, mh, :])
                fvT_q[jg] = fvT

            def emit_K(jg):
                fvT = fvT_q.pop(jg)
                t_sb = [work.tile([128, JW], f16, tag=f"t{ah}", name=f"t{ah}")
                        for ah in range(2)]
                for ah in range(2):
                    psk = ps_k.tile([128, JW], f32, tag="kk", name="kk")
                    for mh in range(2):
                        nc.tensor.matmul(
                            psk, lhsT=UaT[mh][:, ah * 128:(ah + 1) * 128],
                            rhs=fvT[mh], start=(mh == 0), stop=(mh == 1))
                    nc.scalar.activation(out=t_sb[ah], in_=psk, func=AF.Tanh,
                                         bias=qT[ah][:, b:b + 1], scale=1.0)
                t_q[jg] = t_sb

            def emit_E(jg):
                t_sb = t_q.pop(jg)
                pse = ps_e.tile([1, JW], f32, tag="ee", name="ee")
                for ah in range(2):
                    nc.tensor.matmul(pse, lhsT=w_sb[ah], rhs=t_sb[ah],
                                     start=(ah == 0), stop=(ah == 1))
                nc.vector.tensor_copy(out=e_sb[:, jg * JW:(jg + 1) * JW],
                                      in_=pse)

            for i in range(NJG + 2):
                if i < NJG:
                    emit_T(i)
                if 1 <= i:
                    emit_K(i - 1) if (i - 1) < NJG else None
                if 2 <= i:
                    emit_E(i - 2)

            # softmax pieces: e [1, 4096] -> e_t [128, 32] with
            # e_t[p, t] = e[128*t + p] (partition scatter via DRAM bounce)
            e_t = small.tile([128, NL], f32, tag="e_t", name="e_t")
            e_d = dram.tile([L], f32, tag="e_d", name="e_d")
            nc.sync.dma_start(
                out=bass.AP(tensor=e_d.tensor, offset=e_d.offset,
                            ap=[[0, 1], [1, L]]),
                in_=e_sb)
            nc.sync.dma_start(
                out=e_t,
                in_=bass.AP(tensor=e_d.tensor, offset=e_d.offset,
                            ap=[[1, 128], [128, NL]]))
            mrow = small.tile([128, 1], f32, tag="mrow", name="mrow")
            nc.vector.reduce_max(out=mrow, in_=e_t, axis=mybir.AxisListType.X)
            mall = small.tile([128, 1], f32, tag="mall", name="mall")
            nc.gpsimd.partition_all_reduce(mall, mrow, channels=128,
                                           reduce_op=bass_isa.ReduceOp.max)
            negm = small.tile([128, 1], f32, tag="negm", name="negm")
            nc.vector.tensor_scalar_mul(negm, mall, -1.0)
            p_t = small.tile([128, NL], f16, tag="p_t", name="p_t")
            srow = small.tile([128, 1], f32, tag="srow", name="srow")
            nc.scalar.activation(out=p_t, in_=e_t, func=AF.Exp, bias=negm,
                                 scale=1.0, accum_out=srow)
            sall = small.tile([128, 1], f32, tag="sall", name="sall")
            nc.gpsimd.partition_all_reduce(sall, srow, channels=128,
                                           reduce_op=bass_isa.ReduceOp.add)
            rz = small.tile([1, 1], f32, tag="rz", name="rz")
            nc.vector.reciprocal(out=rz, in_=sall[0:1, :])
            return fv_nat, p_t, rz

        # weighted sum, one batch row behind
        def ws_stage(b, fv_nat, p_t, rz):
            psw = ps_mm.tile([1, M], f32, tag="mm", name="mm")
            for t in range(NL):
                nc.tensor.matmul(psw, lhsT=p_t[:, t:t + 1], rhs=fv_nat[:, t, :],
                                 start=(t == 0), stop=(t == NL - 1))
            ctxs = small.tile([1, M], f32, tag="ctx", name="ctx")
            nc.vector.tensor_scalar_mul(ctxs, psw, rz)
            nc.sync.dma_start(out=ctx_out[b:b + 1, :], in_=ctxs)

        pending = None
        for b in range(BLOC):
            saved = main_stage(b)
            if pending is not None:
                ws_stage(pending[0], *pending[1])
            pending = (b, saved)
        ws_stage(pending[0], *pending[1])

    nc.compile()
    return nc


def _get_nc():
    if "nc" not in _CACHE:
        _CACHE["nc"] = _build()
    return _CACHE["nc"]


def kernel(hidden_state, feature_vectors, Wa, Ua, w, ba):
    from concourse.bass_utils import run_bass_kernel_spmd

    nc = _get_nc()
    hidden_state = np.ascontiguousarray(hidden_state, dtype=np.float32)
    feature_vectors = np.ascontiguousarray(feature_vectors, dtype=np.float32)
    params = {
        "Wa": np.ascontiguousarray(Wa, dtype=np.float32),
        "Ua": np.ascontiguousarray(Ua, dtype=np.float32),
        "w": np.ascontiguousarray(w, dtype=np.float32),
        "ba": np.ascontiguousarray(ba, dtype=np.float32),
    }
    in_maps = [
        {
            "hidden_state": hidden_state[c * BLOC:(c + 1) * BLOC],
            "feature_vectors": feature_vectors[c * BLOC:(c + 1) * BLOC],
            **params,
        }
        for c in range(NCORES)
    ]
    res = run_bass_kernel_spmd(nc, in_maps, list(range(NCORES)))
    return np.concatenate([res.results[c]["context"] for c in range(NCORES)],
                          axis=0)


# revision 10
# speedup vs baseline: 1.1106x; 1.0055x over previous
"""Bass/Tile Trainium2 kernel for the additive-attention module.

reference (per batch row b):
    q = hidden_state @ Wa.T + ba                 # [A]
    k = feature_vectors[b] @ Ua.T                # [L, A]
    e = tanh(q + k) @ w                          # [L]
    attn = softmax(e)                            # [L]
    context[b] = attn @ feature_vectors[b]       # [M]

Sharding: data-parallel over batch B=64 -> 8 cores x 8 rows, params
replicated, no collectives. Each core streams its 32 MB feature_vector
shard from HBM exactly once.

Precision: fp32 matmuls cost 4 cycles/column on TRN2 PE (two half-rate
passes); fp16 costs 1. The fv pipeline (fv, Ua, tanh output, attn
weights) runs in fp16 (10 mantissa bits; measured end-to-end rel err
2.7e-3 vs 1.8e-2 for bf16); softmax statistics and all accumulations
(PSUM) stay fp32.

Per-core dataflow (per batch row):
  - fv cast fp32->fp16 during the HBM DMA (SWDGE), natural [l, m] layout
  - PE transposes 128x128 fp16 tiles of fv into [m, l] layout (PSUM),
    DVE evacuates at 16-bit 2x mode; k-matmul streams fv.T columns with
    Ua.T stationary
  - ScalarE evacuates the k PSUM with fused per-partition bias q[a] and
    tanh in one ACTIVATE (fp16 out)
  - e = w.T @ tanh(...) on PE; softmax: DRAM-bounce reshape of e
    [1,4096] -> [128,32], DVE row max, GPSIMD cross-partition max, ACT
    exp with accum_out row sums, GPSIMD cross-partition sum
  - weighted sum on PE: attn column [128,1] fp16 stationary, natural
    fv tiles streaming; denominator applied at the end in fp32
  - the weighted-sum stage of row b is emitted after row b+1's main
    stage so PE never stalls on the softmax tail
"""

import numpy as np

B, R, M, A, L = 64, 512, 256, 256, 4096
NCORES = 8
BLOC = B // NCORES  # 8 batch rows per core
NL = L // 128  # 32 l-chunks of 128
NJG = 8  # j-groups of 512 l-columns
JW = L // NJG  # 512

_CACHE = {}


def _build():
    from contextlib import ExitStack

    import concourse.bacc as bacc
    import concourse.bass as bass
    import concourse.bass_isa as bass_isa
    import concourse.mybir as mybir
    import concourse.tile as tile
    from concourse.masks import make_identity

    f32 = mybir.dt.float32
    f16 = mybir.dt.float16
    AF = mybir.ActivationFunctionType

    nc = bacc.Bacc("TRN2", target_bir_lowering=False, debug=False,
                   num_devices=NCORES)

    hs = nc.dram_tensor("hidden_state", [BLOC, R], f32, kind="ExternalInput").ap()
    fv = nc.dram_tensor("feature_vectors", [BLOC, L, M], f32,
                        kind="ExternalInput").ap()
    Wa = nc.dram_tensor("Wa", [A, R], f32, kind="ExternalInput").ap()
    Ua = nc.dram_tensor("Ua", [A, M], f32, kind="ExternalInput").ap()
    w = nc.dram_tensor("w", [A, 1], f32, kind="ExternalInput").ap()
    ba = nc.dram_tensor("ba", [1, A], f32, kind="ExternalInput").ap()
    ctx_out = nc.dram_tensor("context", [BLOC, M], f32, kind="ExternalOutput").ap()

    with tile.TileContext(nc) as tc, ExitStack() as ctx:
        singles = ctx.enter_context(tc.tile_pool(name="singles", bufs=1))
        ldpool = ctx.enter_context(tc.tile_pool(name="ldpool", bufs=2))
        fvpool = ctx.enter_context(tc.tile_pool(name="fvpool", bufs=3))
        work = ctx.enter_context(tc.tile_pool(name="work", bufs=3))
        small = ctx.enter_context(tc.tile_pool(name="small", bufs=2))
        ps_tp = ctx.enter_context(tc.tile_pool(name="ps_tp", bufs=2, space="PSUM"))
        ps_k = ctx.enter_context(tc.tile_pool(name="ps_k", bufs=3, space="PSUM"))
        ps_e = ctx.enter_context(tc.tile_pool(name="ps_e", bufs=2, space="PSUM"))
        ps_mm = ctx.enter_context(tc.tile_pool(name="ps_mm", bufs=1, space="PSUM"))
        dram = ctx.enter_context(tc.tile_pool(name="dram", bufs=2, space="DRAM"))

        # batch 0's fv cast-load is issued first (it shares the GPSIMD
        # SWDGE queue with make_identity) and split in two so the first
        # transposes start after ~6 us instead of ~12
        fv_nat0 = fvpool.tile([128, NL, M], f16, tag="fv", name="fv")
        for part in range(2):
            fsrc = bass.AP(tensor=fv.tensor, offset=part * (NL // 2) * 128 * M,
                           ap=[[M, 128], [128 * M, NL // 2], [1, M]])
            nc.gpsimd.dma_start(
                out=fv_nat0[:, part * (NL // 2):(part + 1) * (NL // 2), :],
                in_=fsrc)

        ident = singles.tile([128, 128], f32, tag="ident", name="ident")
        make_identity(nc, ident)
        ident16 = singles.tile([128, 128], f16, tag="ident16", name="ident16")
        make_identity(nc, ident16)

        # ---- parameters into contraction-major layouts ----
        # WaT[rt] [128(r), 256(a)] fp32: WaT[rt][k, a] = Wa[a, 128*rt + k]
        WaT = [singles.tile([128, A], f32, tag=f"WaT{rt}", name=f"WaT{rt}")
               for rt in range(4)]
        for at in range(2):
            wa_nat = ldpool.tile([128, R], f32, tag="ld", name="ld")
            nc.sync.dma_start(out=wa_nat, in_=Wa[at * 128:(at + 1) * 128, :])
            for rt in range(4):
                ps = ps_mm.tile([128, 128], f32, tag="mm", name="mm")
                nc.tensor.transpose(ps, wa_nat[:, rt * 128:(rt + 1) * 128], ident)
                nc.vector.tensor_copy(out=WaT[rt][:, at * 128:(at + 1) * 128],
                                      in_=ps)
        # UaT[mh] [128(m), 256(a)] fp16: UaT[mh][k, a] = Ua[a, 128*mh + k]
        UaT = [singles.tile([128, A], f16, tag=f"UaT{mh}", name=f"UaT{mh}")
               for mh in range(2)]
        for at in range(2):
            ua_nat = ldpool.tile([128, M], f32, tag="ld", name="ld")
            nc.sync.dma_start(out=ua_nat, in_=Ua[at * 128:(at + 1) * 128, :])
            for mh in range(2):
                ps = ps_mm.tile([128, 128], f32, tag="mm", name="mm")
                nc.tensor.transpose(ps, ua_nat[:, mh * 128:(mh + 1) * 128], ident)
                nc.vector.tensor_copy(out=UaT[mh][:, at * 128:(at + 1) * 128],
                                      in_=ps)
        # w as fp16 stationary columns [128, 1] per a-half (cast during DMA)
        w_sb = [singles.tile([128, 1], f16, tag=f"w{ah}", name=f"w{ah}")
                for ah in range(2)]
        for ah in range(2):
            nc.gpsimd.dma_start(out=w_sb[ah], in_=w[ah * 128:(ah + 1) * 128, :])

        # hsT[rt] [128(r), BLOC] fp32
        hsT = [singles.tile([128, BLOC], f32, tag=f"hsT{rt}", name=f"hsT{rt}")
               for rt in range(4)]
        for rt in range(4):
            src = bass.AP(tensor=hs.tensor, offset=rt * 128,
                          ap=[[1, 128], [R, BLOC]])
            nc.sync.dma_start(out=hsT[rt], in_=src)

        # q = hs @ Wa.T + ba   -> [BLOC, A] fp32
        q_ps = ps_mm.tile([BLOC, A], f32, tag="mm", name="mm")
        for rt in range(4):
            nc.tensor.matmul(q_ps, lhsT=hsT[rt], rhs=WaT[rt],
                             start=(rt == 0), stop=(rt == 3))
        ba_b = singles.tile([BLOC, A], f32, tag="ba", name="ba")
        nc.sync.dma_start(out=ba_b,
                          in_=bass.AP(tensor=ba.tensor, offset=0,
                                      ap=[[0, BLOC], [1, A]]))
        q_sb = singles.tile([BLOC, A], f32, tag="q", name="q")
        nc.vector.tensor_add(q_sb, q_ps, ba_b)
        # qT[ah] [128(a), BLOC] fp32
        qT = [singles.tile([128, BLOC], f32, tag=f"qT{ah}", name=f"qT{ah}")
              for ah in range(2)]
        for ah in range(2):
            ps = ps_mm.tile([128, BLOC], f32, tag="mm", name="mm")
            nc.tensor.transpose(ps, q_sb[:, ah * 128:(ah + 1) * 128],
                                ident[:BLOC, :BLOC])
            nc.vector.tensor_copy(out=qT[ah], in_=ps)

        # ---- main per-batch-row pipeline ----
        def main_stage(b):
            if b == 0:
                fv_nat = fv_nat0
            else:
                fv_nat = fvpool.tile([128, NL, M], f16, tag="fv", name="fv")
                src = bass.AP(tensor=fv.tensor, offset=b * L * M,
                              ap=[[M, 128], [128 * M, NL], [1, M]])
                nc.gpsimd.dma_start(out=fv_nat, in_=src)  # SWDGE cast f32->f16

            e_sb = small.tile([1, L], f32, tag="e_sb", name="e_sb")
            e_d = dram.tile([L], f32, tag="e_d", name="e_d")

            # software-pipelined j-groups: PE emits transposes(i),
            # k-matmuls(i-1), e-matmul(i-2) per step so it never stalls on
            # the DVE fvT-copy or ACT tanh of the current group.
            fvT_q = {}
            t_q = {}

            def emit_T(jg):
                # both m-halves' transposes share one fp16 PSUM bank
                pst = ps_tp.tile([128, 2, JW], f16, tag="tp", name="tp")
                for mh in range(2):
                    for c in range(4):
                        t = jg * 4 + c
                        nc.tensor.transpose(
                            pst[:, mh, c * 128:(c + 1) * 128],
                            fv_nat[:, t, mh * 128:(mh + 1) * 128], ident16)
                fvT = [work.tile([128, JW], f16, tag=f"fvT{mh}", name=f"fvT{mh}")
                       for mh in range(2)]
                for mh in range(2):
                    nc.vector.tensor_copy(out=fvT[mh], in_=pst[:, mh, :])
                fvT_q[jg] = fvT

            def emit_K(jg):
                fvT = fvT_q.pop(jg)
                t_sb = [work.tile([128, JW], f16, tag=f"t{ah}", name=f"t{ah}")
                        for ah in range(2)]
                for ah in range(2):
                    psk = ps_k.tile([128, JW], f32, tag="kk", name="kk")
                    for mh in range(2):
                        nc.tensor.matmul(
                            psk, lhsT=UaT[mh][:, ah * 128:(ah + 1) * 128],
                            rhs=fvT[mh], start=(mh == 0), stop=(mh == 1))
                    nc.scalar.activation(out=t_sb[ah], in_=psk, func=AF.Tanh,
                                         bias=qT[ah][:, b:b + 1], scale=1.0)
                t_q[jg] = t_sb

            def emit_E(jg):
                t_sb = t_q.pop(jg)
                pse = ps_e.tile([1, JW], f32, tag="ee", name="ee")
                for ah in range(2):
                    nc.tensor.matmul(pse, lhsT=w_sb[ah], rhs=t_sb[ah],
                                     start=(ah == 0), stop=(ah == 1))
                nc.vector.tensor_copy(out=e_sb[:, jg * JW:(jg + 1) * JW],
                                      in_=pse)
                nc.sync.dma_start(
                    out=bass.AP(tensor=e_d.tensor, offset=e_d.offset + jg * JW,
                                ap=[[0, 1], [1, JW]]),
                    in_=e_sb[:, jg * JW:(jg + 1) * JW])

            for i in range(NJG + 2):
                if i < NJG:
                    emit_T(i)
                if 1 <= i:
                    emit_K(i - 1) if (i - 1) < NJG else None
                if 2 <= i:
                    emit_E(i - 2)

            # softmax pieces: e [1, 4096] -> e_t [128, 32] with
            # e_t[p, t] = e[128*t + p] (partition scatter via DRAM bounce)
            e_t = small.tile([128, NL], f32, tag="e_t", name="e_t")
            nc.sync.dma_start(
                out=e_t,
                in_=bass.AP(tensor=e_d.tensor, offset=e_d.offset,
                            ap=[[1, 128], [128, NL]]))
            mrow = small.tile([128, 1], f32, tag="mrow", name="mrow")
            nc.vector.reduce_max(out=mrow, in_=e_t, axis=mybir.AxisListType.X)
            mall = small.tile([128, 1], f32, tag="mall", name="mall")
            nc.gpsimd.partition_all_reduce(mall, mrow, channels=128,
                                           reduce_op=bass_isa.ReduceOp.max)
            negm = small.tile([128, 1], f32, tag="negm", name="negm")
            nc.vector.tensor_scalar_mul(negm, mall, -1.0)
            p_t = small.tile([128, NL], f16, tag="p_t", name="p_t")
            srow = small.tile([128, 1], f32, tag="srow", name="srow")
            nc.scalar.activation(out=p_t, in_=e_t, func=AF.Exp, bias=negm,
                                 scale=1.0, accum_out=srow)
            sall = small.tile([128, 1], f32, tag="sall", name="sall")
            nc.gpsimd.partition_all_reduce(sall, srow, channels=128,
                                           reduce_op=bass_isa.ReduceOp.add)
            rz = small.tile([1, 1], f32, tag="rz", name="rz")
            nc.vector.reciprocal(out=rz, in_=sall[0:1, :])
            return fv_nat, p_t, rz

        # weighted sum, one batch row behind
        def ws_stage(b, fv_nat, p_t, rz):
            psw = ps_mm.tile([1, M], f32, tag="mm", name="mm")
            for t in range(NL):
                nc.tensor.matmul(psw, lhsT=p_t[:, t:t + 1], rhs=fv_nat[:, t, :],
                                 start=(t == 0), stop=(t == NL - 1))
            ctxs = small.tile([1, M], f32, tag="ctx", name="ctx")
            nc.vector.tensor_scalar_mul(ctxs, psw, rz)
            nc.sync.dma_start(out=ctx_out[b:b + 1, :], in_=ctxs)

        pending = None
        for b in range(BLOC):
            saved = main_stage(b)
            if pending is not None:
                ws_stage(pending[0], *pending[1])
            pending = (b, saved)
        ws_stage(pending[0], *pending[1])

    nc.compile()
    return nc


def _get_nc():
    if "nc" not in _CACHE:
        _CACHE["nc"] = _build()
    return _CACHE["nc"]


def kernel(hidden_state, feature_vectors, Wa, Ua, w, ba):
    from concourse.bass_utils import run_bass_kernel_spmd

    nc = _get_nc()
    hidden_state = np.ascontiguousarray(hidden_state, dtype=np.float32)
    feature_vectors = np.ascontiguousarray(feature_vectors, dtype=np.float32)
    params = {
        "Wa": np.ascontiguousarray(Wa, dtype=np.float32),
        "Ua": np.ascontiguousarray(Ua, dtype=np.float32),
        "w": np.ascontiguousarray(w, dtype=np.float32),
        "ba": np.ascontiguousarray(ba, dtype=np.float32),
    }
    in_maps = [
        {
            "hidden_state": hidden_state[c * BLOC:(c + 1) * BLOC],
            "feature_vectors": feature_vectors[c * BLOC:(c + 1) * BLOC],
            **params,
        }
        for c in range(NCORES)
    ]
    res = run_bass_kernel_spmd(nc, in_maps, list(range(NCORES)))
    return np.concatenate([res.results[c]["context"] for c in range(NCORES)],
                          axis=0)
